# revision 3
# baseline (speedup 1.0000x reference)
"""Trainium2 Bass kernel for nn_AttentionModel (B=4, S=2048, H=8, D=64).

Sharding: 32 (batch, head) pairs split 4-per-core across 8 NeuronCores
(data + head parallel); the tiny 64x64 projections are folded into
host-side constants, so each core runs pure per-head attention.

Device pipeline (all matmuls bf16, K=128 zero-padded, N=512):
  scores = x̃q^T M̃ x̃k  with  M̃ = [[Wq^T Wk, Wq^T bk], [bq^T Wk, bq.bk]]
  (65x65, host-computed, shared across heads; x̃ = [x; 1]) — this folds
  BOTH q/k projections + biases into one stationary operand.
  ỹ^T[h] = M̃^T x̃q^T[h]  (PE) -> SBUF bf16 [65, S]-in-[128, S] (padded).
  scores^T[j, i] = x̃k^T_jtile^T @ ỹ^T   (PE, zero-padded K).
  exp(s/8) fused with PSUM evacuation: one wide ACT Exp per PSUM strip.
  PV: stationary [x_v | 1] j-tiles -> acc[65, S] in PSUM accumulates
  U^T with the softmax denominator in row 64; one wide ACT copy/head.
  Output ships unnormalized U^T|den [h, 65, S] bf16; the host applies
  the 64x64 V projection + bias, divides by den, and transposes.
"""
import numpy as np

B, S, H, D = 4, 2048, 8, 64
NCORES = 8
HPC = 4            # heads per core
NJ = 16            # key tiles of 128
IC = 512           # query-chunk width
NCH = S // IC      # 4 chunks
EXPW = 1024        # exp strip width (PSUM banks per strip = EXPW/512)
NBUF = 2           # strip double-buffering

_cache = {}


def _build(repeat=1):
    import concourse.bacc as bacc
    import concourse.mybir as mybir
    from concourse.tile import TileContext
    from concourse.bass import ts

    F32 = mybir.dt.float32
    BF16 = mybir.dt.bfloat16
    AF = mybir.ActivationFunctionType

    nc = bacc.Bacc("TRN2", target_bir_lowering=False, debug=False,
                   num_devices=NCORES)

    xq = nc.declare_dram_parameter("xq", [HPC, 65, S], BF16, isOutput=False)
    xk = nc.declare_dram_parameter("xk", [HPC, 65, S], BF16, isOutput=False)
    vp = nc.declare_dram_parameter("vp", [HPC, 128, NJ * 65], BF16,
                                   isOutput=False)
    m65 = nc.declare_dram_parameter("m65", [128, 65], BF16, isOutput=False)
    out_dr = nc.declare_dram_parameter("out", [HPC, 65, S], BF16,
                                       isOutput=True)

    NSTRIP = EXPW // IC            # j-tiles (matmuls) per exp strip
    NROUND = NJ // NSTRIP          # exp strips per (head, chunk)

    with TileContext(nc) as tc:
        with (
            tc.tile_pool(name="constp", bufs=1) as constp,
            tc.tile_pool(name="esbp", bufs=2) as esbp,
            tc.tile_pool(name="psbig", bufs=NBUF, space="PSUM") as psbig,
            tc.tile_pool(name="psacc", bufs=1, space="PSUM") as psacc,
        ):
            xq_sb, xk_sb, vp_sb, y_sb, u_sb = [], [], [], [], []
            for h in range(HPC):
                t = constp.tile([128, S], BF16, name=f"xq{h}")
                nc.gpsimd.memset(t[64:128, :], 0.0)
                xq_sb.append(t)
                t = constp.tile([128, S], BF16, name=f"xk{h}")
                nc.gpsimd.memset(t[64:128, :], 0.0)
                xk_sb.append(t)
                t = constp.tile([128, NJ * 65], BF16, name=f"vp{h}")
                vp_sb.append(t)
                t = constp.tile([128, S], BF16, name=f"y{h}")
                nc.gpsimd.memset(t[64:128, :], 0.0)
                y_sb.append(t)
                t = constp.tile([65, S], BF16, name=f"u{h}")
                u_sb.append(t)
            m65_sb = constp.tile([128, 65], BF16, name="m65")

            for rep in range(repeat):
                # per-call input loads, spread across DMA queues
                for h in range(HPC):
                    eng = nc.sync if h % 2 == 0 else nc.scalar
                    oth = nc.scalar if h % 2 == 0 else nc.sync
                    eng.dma_start(xq_sb[h][0:65, :], xq[h, :, :])
                    oth.dma_start(xk_sb[h][0:65, :], xk[h, :, :])
                    nc.gpsimd.dma_start(vp_sb[h][:], vp[h, :, :])
                nc.scalar.dma_start(m65_sb[:], m65[:, :])

                # ---- projections: ỹ^T[h] = m65.T @ x̃q^T[h] ----
                for h in range(HPC):
                    pp = psacc.tile([65, S], F32,
                                    name=f"pp{h}_{rep}", tag="acc")
                    for c in range(NCH):
                        nc.tensor.matmul(pp[:, ts(c, IC)], m65_sb[:],
                                         xq_sb[h][:, ts(c, IC)],
                                         start=True, stop=True)
                    nc.scalar.copy(y_sb[h][0:65, :], pp[:])

                # ---- attention ----
                for h in range(HPC):
                    acc4 = psacc.tile([65, S], F32,
                                      name=f"acc{h}_{rep}", tag="acc")
                    for c in range(NCH):
                        esb = esbp.tile([128, NJ * IC], BF16,
                                        name=f"esb{h}_{c}_{rep}", tag="esb")
                        for r in range(NROUND):
                            bigp = psbig.tile([128, EXPW], F32,
                                              name=f"bp{h}_{c}_{r}_{rep}",
                                              tag="bigp")
                            for u in range(NSTRIP):
                                jt = r * NSTRIP + u
                                nc.tensor.matmul(
                                    bigp[:, ts(u, IC)],
                                    xk_sb[h][:, ts(jt, 128)],
                                    y_sb[h][:, ts(c, IC)],
                                    start=True, stop=True)
                            nc.scalar.activation(esb[:, ts(r, EXPW)],
                                                 bigp[:], AF.Exp, scale=0.125)
                        for jt in range(NJ):
                            nc.tensor.matmul(acc4[:, ts(c, IC)],
                                             vp_sb[h][:, jt * 65:jt * 65 + 65],
                                             esb[:, ts(jt, IC)],
                                             start=(jt == 0),
                                             stop=(jt == NJ - 1))
                    nc.scalar.copy(u_sb[h][:, :], acc4[:])
                    eng = nc.sync if h % 2 == 0 else nc.scalar
                    eng.dma_start(out_dr[h, :, :], u_sb[h][:])

    nc.compile()
    return nc


def _bf16_bits(x):
    """f32 ndarray -> uint16 bf16 bits, round-to-nearest-even (vectorized)."""
    u = np.ascontiguousarray(x, np.float32).view(np.uint32)
    return ((u + 0x7FFF + ((u >> 16) & 1)) >> 16).astype(np.uint16)


_ONE_BF16 = np.uint16(0x3F80)


def _prep_inputs(query, key, value, Wq, bq, Wk, bk, Wv, bv):
    """Host-side sharding/layout prep. Returns per-core input maps.

    All bf16 tensors are built as uint16 bit patterns (fast vectorized
    cast) and viewed as ml_dtypes.bfloat16 at the end.
    """
    import ml_dtypes
    BF = ml_dtypes.bfloat16
    f32 = np.float32

    qu = _bf16_bits(query)                 # [B, S, H, D] u16
    ku = _bf16_bits(key)
    vu = _bf16_bits(value)
    Wq, bq = np.asarray(Wq, f32), np.asarray(bq, f32)
    Wk, bk = np.asarray(Wk, f32), np.asarray(bk, f32)

    def padT(xu):  # [B,S,H,D] u16 -> x̃^T [bh, 65, S] u16
        o = np.empty((B * H, 65, S), np.uint16)
        o[:, 0:64, :] = xu.transpose(0, 2, 3, 1).reshape(B * H, D, S)
        o[:, 64, :] = _ONE_BF16
        return o

    xqt, xkt = padT(qu), padT(ku)

    # vp [bh, 128, NJ*65]: [x_v | 1] per j-tile, partition-major
    vpk = np.empty((B * H, 128, NJ, 65), np.uint16)
    vpk[:, :, :, 0:64] = vu.reshape(B, NJ, 128, H, D) \
        .transpose(0, 3, 2, 1, 4).reshape(B * H, 128, NJ, D)
    vpk[:, :, :, 64] = _ONE_BF16
    vpk = vpk.reshape(B * H, 128, NJ * 65)

    m = np.zeros((128, 65), f32)
    m[0:64, 0:64] = Wq.T @ Wk
    m[0:64, 64] = Wq.T @ bk
    m[64, 0:64] = bq @ Wk
    m[64, 64] = bq @ bk
    m65 = _bf16_bits(m).view(BF)

    in_maps = []
    for c in range(NCORES):
        sl = slice(c * HPC, (c + 1) * HPC)
        in_maps.append(dict(
            xq=np.ascontiguousarray(xqt[sl]).view(BF),
            xk=np.ascontiguousarray(xkt[sl]).view(BF),
            vp=np.ascontiguousarray(vpk[sl]).view(BF),
            m65=m65))
    return in_maps


def _make_runner(nc):
    """Cached sharded-jit runner; donated output buffers created on device."""
    import jax
    import jax.numpy as jnp
    from jax.sharding import Mesh, PartitionSpec, NamedSharding
    from jax.experimental.shard_map import shard_map
    from concourse import mybir
    from concourse.bass2jax import (_bass_exec_p, partition_id_tensor,
                                    install_neuronx_cc_hook)

    install_neuronx_cc_hook()
    partition_name = (nc.partition_id_tensor.name
                      if nc.partition_id_tensor else None)
    in_names, out_names, out_shapes, out_dtypes = [], [], [], []
    for alloc in nc.m.functions[0].allocations:
        if not isinstance(alloc, mybir.MemoryLocationSet):
            continue
        name = alloc.memorylocations[0].name
        if alloc.kind == "ExternalInput":
            if name != partition_name:
                in_names.append(name)
        elif alloc.kind == "ExternalOutput":
            out_names.append(name)
            out_shapes.append(tuple(alloc.tensor_shape))
            out_dtypes.append(mybir.dt.np(alloc.dtype))

    n_params = len(in_names)
    n_outs = len(out_names)
    out_avals = [jax.core.ShapedArray(s, d)
                 for s, d in zip(out_shapes, out_dtypes)]
    all_in_names = tuple(in_names + out_names +
                         ([partition_name] if partition_name else []))
    donate = tuple(range(n_params, n_params + n_outs))

    def _body(*args):
        operands = list(args)
        if partition_name is not None:
            operands.append(partition_id_tensor())
        outs = _bass_exec_p.bind(
            *operands, out_avals=tuple(out_avals), in_names=all_in_names,
            out_names=tuple(out_names), lowering_input_output_aliases=(),
            sim_require_finite=True, sim_require_nnan=True, nc=nc)
        return tuple(outs)

    devices = jax.devices()[:NCORES]
    mesh = Mesh(np.asarray(devices), ("core",))
    in_specs = (PartitionSpec("core"),) * (n_params + n_outs)
    out_specs = (PartitionSpec("core"),) * n_outs
    sharded = jax.jit(
        shard_map(_body, mesh=mesh, in_specs=in_specs,
                  out_specs=out_specs, check_rep=False),
        donate_argnums=donate, keep_unused=True)

    shard = NamedSharding(mesh, PartitionSpec("core"))
    zeros_fn = jax.jit(
        lambda: tuple(jnp.zeros((NCORES * s[0], *s[1:]), d)
                      for s, d in zip(out_shapes, out_dtypes)),
        out_shardings=(shard,) * n_outs)

    def run(in_maps):
        concat_in = [
            np.concatenate([np.asarray(in_maps[c][name])
                            for c in range(NCORES)], axis=0)
            for name in in_names]
        outs = sharded(*concat_in, *zeros_fn())
        np_outs = [np.asarray(o) for o in outs]
        return [
            {name: np_outs[i].reshape(NCORES, *out_shapes[i])[c]
             for i, name in enumerate(out_names)}
            for c in range(NCORES)]

    return run


def _postprocess(res, Wv, bv):
    """res: per-core result dicts -> full [B, S, H, D] f32 output."""
    f32 = np.float32
    Wv, bv = np.asarray(Wv, f32), np.asarray(bv, f32)
    outs = np.stack([np.asarray(res[c]["out"], f32)
                     for c in range(NCORES)])          # [8, 4, 65, S]
    outs = outs.reshape(B * H, 65, S)
    U = outs[:, 0:64, :]                               # [bh, d, s]
    den = outs[:, 64:65, :]                            # [bh, 1, s]
    o = np.einsum('hds,ed->hse', U, Wv, optimize=True) / den.transpose(0, 2, 1)
    o = o + bv                                         # [bh, s, e]
    o = o.reshape(B, H, S, D).transpose(0, 2, 1, 3)    # [B, S, H, D]
    return np.ascontiguousarray(o.astype(f32))


def kernel(query, key, value, Wq, bq, Wk, bk, Wv, bv):
    if "nc" not in _cache:
        _cache["nc"] = _build()
    nc = _cache["nc"]

    in_maps = _prep_inputs(query, key, value, Wq, bq, Wk, bk, Wv, bv)
    try:
        if "run" not in _cache:
            _cache["run"] = _make_runner(nc)
        res = _cache["run"](in_maps)
    except Exception:
        from concourse.bass_utils import run_bass_kernel_spmd
        res = run_bass_kernel_spmd(nc, in_maps, list(range(NCORES))).results
    return _postprocess(res, Wv, bv)


# revision 7
# speedup vs baseline: 3.6891x; 3.6891x over previous
"""Trainium2 Bass kernel for nn_AttentionModel (B=4, S=2048, H=8, D=64).

Sharding: 32 (batch, head) pairs split 4-per-core across 8 NeuronCores
(data + head parallel); the tiny 64x64 projections are folded into
host-side constants, so each core runs pure per-head attention.

Device pipeline (all matmuls bf16, K=128 zero-padded, N=512):
  scores = x̃q^T M̃ x̃k  with  M̃ = [[Wq^T Wk, Wq^T bk], [bq^T Wk, bq.bk]]
  (65x65, host-computed, shared across heads; x̃ = [x; 1]) — this folds
  BOTH q/k projections + biases into one stationary operand.
  ỹ^T[h] = M̃^T x̃q^T[h]  (PE) -> SBUF bf16 [65, S]-in-[128, S] (padded).
  scores^T[j, i] = x̃k^T_jtile^T @ ỹ^T   (PE, zero-padded K).
  exp(s/8) fused with PSUM evacuation: one wide ACT Exp per PSUM strip.
  PV: stationary [x_v | 1] j-tiles -> acc[65, S] in PSUM accumulates
  U^T with the softmax denominator in row 64; one wide ACT copy/head.
  Output ships unnormalized U^T|den [h, 65, S] bf16; the host applies
  the 64x64 V projection + bias, divides by den, and transposes.
"""
import numpy as np

B, S, H, D = 4, 2048, 8, 64
NCORES = 8
HPC = 4            # heads per core
NJ = 16            # key tiles of 128
IC = 512           # query-chunk width
NCH = S // IC      # 4 chunks
EXPW = 1024        # exp strip width (PSUM banks per strip = EXPW/512)
NBUF = 2           # strip double-buffering

_cache = {}


def _build(repeat=1):
    import concourse.bacc as bacc
    import concourse.mybir as mybir
    from concourse.tile import TileContext
    from concourse.bass import ts

    F32 = mybir.dt.float32
    BF16 = mybir.dt.bfloat16
    AF = mybir.ActivationFunctionType

    nc = bacc.Bacc("TRN2", target_bir_lowering=False, debug=False,
                   num_devices=NCORES)

    xq = nc.declare_dram_parameter("xq", [HPC, 65, S], BF16, isOutput=False)
    xk = nc.declare_dram_parameter("xk", [HPC, 65, S], BF16, isOutput=False)
    vp = nc.declare_dram_parameter("vp", [HPC, 128, NJ * 65], BF16,
                                   isOutput=False)
    m65 = nc.declare_dram_parameter("m65", [128, 65], BF16, isOutput=False)
    out_dr = nc.declare_dram_parameter("out", [HPC, 65, S], BF16,
                                       isOutput=True)

    NSTRIP = EXPW // IC            # j-tiles (matmuls) per exp strip
    NROUND = NJ // NSTRIP          # exp strips per (head, chunk)

    with TileContext(nc) as tc:
        with (
            tc.tile_pool(name="constp", bufs=1) as constp,
            tc.tile_pool(name="esbp", bufs=2) as esbp,
            tc.tile_pool(name="psbig", bufs=NBUF, space="PSUM") as psbig,
            tc.tile_pool(name="psacc", bufs=1, space="PSUM") as psacc,
        ):
            xq_sb, xk_sb, vp_sb, y_sb, u_sb = [], [], [], [], []
            for h in range(HPC):
                t = constp.tile([128, S], BF16, name=f"xq{h}")
                nc.gpsimd.memset(t[64:128, :], 0.0)
                xq_sb.append(t)
                t = constp.tile([128, S], BF16, name=f"xk{h}")
                nc.gpsimd.memset(t[64:128, :], 0.0)
                xk_sb.append(t)
                t = constp.tile([128, NJ * 65], BF16, name=f"vp{h}")
                vp_sb.append(t)
                t = constp.tile([128, S], BF16, name=f"y{h}")
                nc.gpsimd.memset(t[64:128, :], 0.0)
                y_sb.append(t)
                t = constp.tile([65, S], BF16, name=f"u{h}")
                u_sb.append(t)
            m65_sb = constp.tile([128, 65], BF16, name="m65")

            for rep in range(repeat):
                # per-call input loads, spread across DMA queues
                for h in range(HPC):
                    eng = nc.sync if h % 2 == 0 else nc.scalar
                    oth = nc.scalar if h % 2 == 0 else nc.sync
                    eng.dma_start(xq_sb[h][0:65, :], xq[h, :, :])
                    oth.dma_start(xk_sb[h][0:65, :], xk[h, :, :])
                    nc.gpsimd.dma_start(vp_sb[h][:], vp[h, :, :])
                nc.scalar.dma_start(m65_sb[:], m65[:, :])

                # ---- projections: ỹ^T[h] = m65.T @ x̃q^T[h] ----
                for h in range(HPC):
                    pp = psacc.tile([65, S], F32,
                                    name=f"pp{h}_{rep}", tag="acc")
                    for c in range(NCH):
                        nc.tensor.matmul(pp[:, ts(c, IC)], m65_sb[:],
                                         xq_sb[h][:, ts(c, IC)],
                                         start=True, stop=True)
                    nc.scalar.copy(y_sb[h][0:65, :], pp[:])

                # ---- attention ----
                for h in range(HPC):
                    acc4 = psacc.tile([65, S], F32,
                                      name=f"acc{h}_{rep}", tag="acc")
                    for c in range(NCH):
                        esb = esbp.tile([128, NJ * IC], BF16,
                                        name=f"esb{h}_{c}_{rep}", tag="esb")
                        for r in range(NROUND):
                            bigp = psbig.tile([128, EXPW], F32,
                                              name=f"bp{h}_{c}_{r}_{rep}",
                                              tag="bigp")
                            for u in range(NSTRIP):
                                jt = r * NSTRIP + u
                                nc.tensor.matmul(
                                    bigp[:, ts(u, IC)],
                                    xk_sb[h][:, ts(jt, 128)],
                                    y_sb[h][:, ts(c, IC)],
                                    start=True, stop=True)
                            nc.scalar.activation(esb[:, ts(r, EXPW)],
                                                 bigp[:], AF.Exp, scale=0.125)
                        for jt in range(NJ):
                            nc.tensor.matmul(acc4[:, ts(c, IC)],
                                             vp_sb[h][:, jt * 65:jt * 65 + 65],
                                             esb[:, ts(jt, IC)],
                                             start=(jt == 0),
                                             stop=(jt == NJ - 1))
                    nc.scalar.copy(u_sb[h][:, :], acc4[:])
                    eng = nc.sync if h % 2 == 0 else nc.scalar
                    eng.dma_start(out_dr[h, :, :], u_sb[h][:])

    nc.compile()
    return nc


def _bf16_bits(x):
    """f32 ndarray -> uint16 bf16 bits, round-to-nearest-even (vectorized)."""
    u = np.ascontiguousarray(x, np.float32).view(np.uint32)
    return ((u + 0x7FFF + ((u >> 16) & 1)) >> 16).astype(np.uint16)


_ONE_BF16 = np.uint16(0x3F80)


def _prep_global(query, key, value, Wq, bq, Wk, bk, Wv, bv):
    """Host-side layout prep -> global (cores-concatenated) input arrays.

    All bf16 tensors are built as uint16 bit patterns (fast vectorized
    cast) and viewed as ml_dtypes.bfloat16 at the end.
    """
    import ml_dtypes
    BF = ml_dtypes.bfloat16
    f32 = np.float32

    qu = _bf16_bits(query)                 # [B, S, H, D] u16
    ku = _bf16_bits(key)
    vu = _bf16_bits(value)
    Wq, bq = np.asarray(Wq, f32), np.asarray(bq, f32)
    Wk, bk = np.asarray(Wk, f32), np.asarray(bk, f32)

    def padT(xu):  # [B,S,H,D] u16 -> x̃^T [bh, 65, S] u16
        o = np.empty((B * H, 65, S), np.uint16)
        o[:, 0:64, :] = xu.transpose(0, 2, 3, 1).reshape(B * H, D, S)
        o[:, 64, :] = _ONE_BF16
        return o

    xqt, xkt = padT(qu), padT(ku)

    # vp [bh, 128, NJ*65]: [x_v | 1] per j-tile, partition-major
    vpk = np.empty((B * H, 128, NJ, 65), np.uint16)
    vpk[:, :, :, 0:64] = vu.reshape(B, NJ, 128, H, D) \
        .transpose(0, 3, 2, 1, 4).reshape(B * H, 128, NJ, D)
    vpk[:, :, :, 64] = _ONE_BF16
    vpk = vpk.reshape(B * H, 128, NJ * 65)

    m = np.zeros((128, 65), f32)
    m[0:64, 0:64] = Wq.T @ Wk
    m[0:64, 64] = Wq.T @ bk
    m[64, 0:64] = bq @ Wk
    m[64, 64] = bq @ bk
    m65 = _bf16_bits(m).view(BF)

    # global (concatenated-over-cores) arrays: per-core shard c is rows
    # [c*HPC:(c+1)*HPC] — already contiguous, no per-core copies needed
    return dict(xq=xqt.view(BF), xk=xkt.view(BF), vp=vpk.view(BF),
                m65=np.ascontiguousarray(np.tile(m65, (NCORES, 1))))


def _prep_inputs(query, key, value, Wq, bq, Wk, bk, Wv, bv):
    """Per-core input maps (fallback / external-harness path)."""
    g = _prep_global(query, key, value, Wq, bq, Wk, bk, Wv, bv)
    in_maps = []
    for c in range(NCORES):
        sl = slice(c * HPC, (c + 1) * HPC)
        in_maps.append(dict(
            xq=np.ascontiguousarray(g["xq"][sl]),
            xk=np.ascontiguousarray(g["xk"][sl]),
            vp=np.ascontiguousarray(g["vp"][sl]),
            m65=np.ascontiguousarray(g["m65"][0:128])))
    return in_maps


def _make_runner(nc):
    """Cached sharded-jit runner; donated output buffers created on device."""
    import jax
    import jax.numpy as jnp
    from jax.sharding import Mesh, PartitionSpec, NamedSharding
    from jax.experimental.shard_map import shard_map
    from concourse import mybir
    from concourse.bass2jax import (_bass_exec_p, partition_id_tensor,
                                    install_neuronx_cc_hook)

    install_neuronx_cc_hook()
    partition_name = (nc.partition_id_tensor.name
                      if nc.partition_id_tensor else None)
    in_names, out_names, out_shapes, out_dtypes = [], [], [], []
    for alloc in nc.m.functions[0].allocations:
        if not isinstance(alloc, mybir.MemoryLocationSet):
            continue
        name = alloc.memorylocations[0].name
        if alloc.kind == "ExternalInput":
            if name != partition_name:
                in_names.append(name)
        elif alloc.kind == "ExternalOutput":
            out_names.append(name)
            out_shapes.append(tuple(alloc.tensor_shape))
            out_dtypes.append(mybir.dt.np(alloc.dtype))

    n_params = len(in_names)
    n_outs = len(out_names)
    out_avals = [jax.core.ShapedArray(s, d)
                 for s, d in zip(out_shapes, out_dtypes)]
    all_in_names = tuple(in_names + out_names +
                         ([partition_name] if partition_name else []))
    donate = tuple(range(n_params, n_params + n_outs))

    def _body(*args):
        operands = list(args)
        if partition_name is not None:
            operands.append(partition_id_tensor())
        outs = _bass_exec_p.bind(
            *operands, out_avals=tuple(out_avals), in_names=all_in_names,
            out_names=tuple(out_names), lowering_input_output_aliases=(),
            sim_require_finite=True, sim_require_nnan=True, nc=nc)
        return tuple(outs)

    devices = jax.devices()[:NCORES]
    mesh = Mesh(np.asarray(devices), ("core",))
    in_specs = (PartitionSpec("core"),) * (n_params + n_outs)
    out_specs = (PartitionSpec("core"),) * n_outs
    sharded = jax.jit(
        shard_map(_body, mesh=mesh, in_specs=in_specs,
                  out_specs=out_specs, check_rep=False),
        donate_argnums=donate, keep_unused=True)

    shard = NamedSharding(mesh, PartitionSpec("core"))
    zeros_fn = jax.jit(
        lambda: tuple(jnp.zeros((NCORES * s[0], *s[1:]), d)
                      for s, d in zip(out_shapes, out_dtypes)),
        out_shardings=(shard,) * n_outs)

    def run(in_maps):
        if isinstance(in_maps, dict):  # fast path: pre-concatenated globals
            concat_in = [in_maps[name] for name in in_names]
        else:
            concat_in = [
                np.concatenate([np.asarray(in_maps[c][name])
                                for c in range(NCORES)], axis=0)
                for name in in_names]
        outs = sharded(*concat_in, *zeros_fn())
        np_outs = [np.asarray(o) for o in outs]
        return [
            {name: np_outs[i].reshape(NCORES, *out_shapes[i])[c]
             for i, name in enumerate(out_names)}
            for c in range(NCORES)]

    return run


def _postprocess(res, Wv, bv):
    """res: per-core result dicts -> full [B, S, H, D] f32 output."""
    f32 = np.float32
    Wv, bv = np.asarray(Wv, f32), np.asarray(bv, f32)
    bits = np.stack([np.asarray(res[c]["out"]).view(np.uint16)
                     for c in range(NCORES)])          # [8, 4, 65, S] u16
    outs = (bits.astype(np.uint32) << 16).view(f32)    # bf16 -> f32, exact
    outs = outs.reshape(B * H, 65, S)
    U = outs[:, 0:64, :]                               # [bh, d, s]
    den = outs[:, 64:65, :]                            # [bh, 1, s]
    oT = np.matmul(Wv, U)                              # [bh, e, s] (BLAS)
    oT /= den
    oT += bv[:, None]
    o = oT.reshape(B, H, D, S).transpose(0, 3, 1, 2)   # [B, S, H, D]
    return np.ascontiguousarray(o)


def kernel(query, key, value, Wq, bq, Wk, bk, Wv, bv):
    if "nc" not in _cache:
        _cache["nc"] = _build()
    nc = _cache["nc"]

    g = _prep_global(query, key, value, Wq, bq, Wk, bk, Wv, bv)
    try:
        if "run" not in _cache:
            _cache["run"] = _make_runner(nc)
        res = _cache["run"](g)
    except Exception:
        from concourse.bass_utils import run_bass_kernel_spmd
        in_maps = [
            {k: np.ascontiguousarray(v[c * HPC:(c + 1) * HPC])
             if k != "m65" else np.ascontiguousarray(v[0:128])
             for k, v in g.items()}
            for c in range(NCORES)]
        res = run_bass_kernel_spmd(nc, in_maps, list(range(NCORES))).results
    return _postprocess(res, Wv, bv)


# revision 8
# speedup vs baseline: 9.1499x; 2.4802x over previous
"""Trainium2 Bass kernel for nn_AttentionModel (B=4, S=2048, H=8, D=64).

Sharding: 32 (batch, head) pairs split 4-per-core across 8 NeuronCores
(data + head parallel); the tiny 64x64 projections are folded into
host-side constants, so each core runs pure per-head attention.

Device pipeline (all matmuls bf16, K=128 zero-padded, N=512):
  scores = x̃q^T M̃ x̃k  with  M̃ = [[Wq^T Wk, Wq^T bk], [bq^T Wk, bq.bk]]
  (65x65, host-computed, shared across heads; x̃ = [x; 1]) — this folds
  BOTH q/k projections + biases into one stationary operand.
  ỹ^T[h] = M̃^T x̃q^T[h]  (PE) -> SBUF bf16 [65, S]-in-[128, S] (padded).
  scores^T[j, i] = x̃k^T_jtile^T @ ỹ^T   (PE, zero-padded K).
  exp(s/8) fused with PSUM evacuation: one wide ACT Exp per PSUM strip.
  PV: stationary [x_v | 1] j-tiles -> acc[65, S] in PSUM accumulates
  U^T with the softmax denominator in row 64; one wide ACT copy/head.
  Output ships unnormalized U^T|den [h, 65, S] bf16; the host applies
  the 64x64 V projection + bias, divides by den, and transposes.
"""
import numpy as np

B, S, H, D = 4, 2048, 8, 64
NCORES = 8
HPC = 4            # heads per core
NJ = 16            # key tiles of 128
IC = 512           # query-chunk width
NCH = S // IC      # 4 chunks
EXPW = 1024        # exp strip width (PSUM banks per strip = EXPW/512)
NBUF = 2           # strip double-buffering

_cache = {}


def _build(repeat=1):
    import concourse.bacc as bacc
    import concourse.mybir as mybir
    from concourse.tile import TileContext
    from concourse.bass import ts

    F32 = mybir.dt.float32
    BF16 = mybir.dt.bfloat16
    AF = mybir.ActivationFunctionType

    nc = bacc.Bacc("TRN2", target_bir_lowering=False, debug=False,
                   num_devices=NCORES)

    xq = nc.declare_dram_parameter("xq", [HPC, 65, S], BF16, isOutput=False)
    xk = nc.declare_dram_parameter("xk", [HPC, 65, S], BF16, isOutput=False)
    vp = nc.declare_dram_parameter("vp", [HPC, 128, NJ * 65], BF16,
                                   isOutput=False)
    m65 = nc.declare_dram_parameter("m65", [128, 65], BF16, isOutput=False)
    out_dr = nc.declare_dram_parameter("out", [HPC, 65, S], BF16,
                                       isOutput=True)

    NSTRIP = EXPW // IC            # j-tiles (matmuls) per exp strip
    NROUND = NJ // NSTRIP          # exp strips per (head, chunk)

    with TileContext(nc) as tc:
        with (
            tc.tile_pool(name="constp", bufs=1) as constp,
            tc.tile_pool(name="esbp", bufs=2) as esbp,
            tc.tile_pool(name="psbig", bufs=NBUF, space="PSUM") as psbig,
            tc.tile_pool(name="psacc", bufs=1, space="PSUM") as psacc,
        ):
            xq_sb, xk_sb, vp_sb, y_sb, u_sb = [], [], [], [], []
            for h in range(HPC):
                t = constp.tile([128, S], BF16, name=f"xq{h}")
                nc.gpsimd.memset(t[64:128, :], 0.0)
                xq_sb.append(t)
                t = constp.tile([128, S], BF16, name=f"xk{h}")
                nc.gpsimd.memset(t[64:128, :], 0.0)
                xk_sb.append(t)
                t = constp.tile([128, NJ * 65], BF16, name=f"vp{h}")
                vp_sb.append(t)
                t = constp.tile([128, S], BF16, name=f"y{h}")
                nc.gpsimd.memset(t[64:128, :], 0.0)
                y_sb.append(t)
                t = constp.tile([65, S], BF16, name=f"u{h}")
                u_sb.append(t)
            m65_sb = constp.tile([128, 65], BF16, name="m65")

            for rep in range(repeat):
                # per-call input loads: every tensor split into column
                # halves issued on both HWDGE rings in parallel
                for h in range(HPC):
                    for g, eng in enumerate((nc.sync, nc.scalar)):
                        eng.dma_start(xq_sb[h][0:65, g * 1024:(g + 1) * 1024],
                                      xq[h, :, g * 1024:(g + 1) * 1024])
                        eng.dma_start(xk_sb[h][0:65, g * 1024:(g + 1) * 1024],
                                      xk[h, :, g * 1024:(g + 1) * 1024])
                        eng.dma_start(vp_sb[h][:, g * 520:(g + 1) * 520],
                                      vp[h, :, g * 520:(g + 1) * 520])
                nc.scalar.dma_start(m65_sb[:], m65[:, :])

                # ---- projections: ỹ^T[h] = m65.T @ x̃q^T[h] ----
                for h in range(HPC):
                    pp = psacc.tile([65, S], F32,
                                    name=f"pp{h}_{rep}", tag="acc")
                    for c in range(NCH):
                        nc.tensor.matmul(pp[:, ts(c, IC)], m65_sb[:],
                                         xq_sb[h][:, ts(c, IC)],
                                         start=True, stop=True)
                    nc.scalar.copy(y_sb[h][0:65, :], pp[:])

                # ---- attention ----
                for h in range(HPC):
                    acc4 = psacc.tile([65, S], F32,
                                      name=f"acc{h}_{rep}", tag="acc")
                    for c in range(NCH):
                        esb = esbp.tile([128, NJ * IC], BF16,
                                        name=f"esb{h}_{c}_{rep}", tag="esb")
                        for r in range(NROUND):
                            bigp = psbig.tile([128, EXPW], F32,
                                              name=f"bp{h}_{c}_{r}_{rep}",
                                              tag="bigp")
                            for u in range(NSTRIP):
                                jt = r * NSTRIP + u
                                nc.tensor.matmul(
                                    bigp[:, ts(u, IC)],
                                    xk_sb[h][:, ts(jt, 128)],
                                    y_sb[h][:, ts(c, IC)],
                                    start=True, stop=True)
                            nc.scalar.activation(esb[:, ts(r, EXPW)],
                                                 bigp[:], AF.Exp, scale=0.125)
                        for jt in range(NJ):
                            nc.tensor.matmul(acc4[:, ts(c, IC)],
                                             vp_sb[h][:, jt * 65:jt * 65 + 65],
                                             esb[:, ts(jt, IC)],
                                             start=(jt == 0),
                                             stop=(jt == NJ - 1))
                    nc.scalar.copy(u_sb[h][:, :], acc4[:])
                    eng = nc.sync if h % 2 == 0 else nc.scalar
                    eng.dma_start(out_dr[h, :, :], u_sb[h][:])

    nc.compile()
    return nc


def _bf16_bits(x):
    """f32 ndarray -> uint16 bf16 bits, round-to-nearest-even (vectorized)."""
    u = np.ascontiguousarray(x, np.float32).view(np.uint32)
    return ((u + 0x7FFF + ((u >> 16) & 1)) >> 16).astype(np.uint16)


_ONE_BF16 = np.uint16(0x3F80)


def _prep_global(query, key, value, Wq, bq, Wk, bk, Wv, bv):
    """Host-side layout prep -> global (cores-concatenated) input arrays.

    All bf16 tensors are built as uint16 bit patterns (fast vectorized
    cast) and viewed as ml_dtypes.bfloat16 at the end.
    """
    import ml_dtypes
    BF = ml_dtypes.bfloat16
    f32 = np.float32

    qu = _bf16_bits(query)                 # [B, S, H, D] u16
    ku = _bf16_bits(key)
    vu = _bf16_bits(value)
    Wq, bq = np.asarray(Wq, f32), np.asarray(bq, f32)
    Wk, bk = np.asarray(Wk, f32), np.asarray(bk, f32)

    def padT(xu):  # [B,S,H,D] u16 -> x̃^T [bh, 65, S] u16
        o = np.empty((B * H, 65, S), np.uint16)
        o[:, 0:64, :] = xu.transpose(0, 2, 3, 1).reshape(B * H, D, S)
        o[:, 64, :] = _ONE_BF16
        return o

    xqt, xkt = padT(qu), padT(ku)

    # vp [bh, 128, NJ*65]: [x_v | 1] per j-tile, partition-major
    vpk = np.empty((B * H, 128, NJ, 65), np.uint16)
    vpk[:, :, :, 0:64] = vu.reshape(B, NJ, 128, H, D) \
        .transpose(0, 3, 2, 1, 4).reshape(B * H, 128, NJ, D)
    vpk[:, :, :, 64] = _ONE_BF16
    vpk = vpk.reshape(B * H, 128, NJ * 65)

    m = np.zeros((128, 65), f32)
    m[0:64, 0:64] = Wq.T @ Wk
    m[0:64, 64] = Wq.T @ bk
    m[64, 0:64] = bq @ Wk
    m[64, 64] = bq @ bk
    m65 = _bf16_bits(m).view(BF)

    # global (concatenated-over-cores) arrays: per-core shard c is rows
    # [c*HPC:(c+1)*HPC] — already contiguous, no per-core copies needed
    return dict(xq=xqt.view(BF), xk=xkt.view(BF), vp=vpk.view(BF),
                m65=np.ascontiguousarray(np.tile(m65, (NCORES, 1))))


def _prep_inputs(query, key, value, Wq, bq, Wk, bk, Wv, bv):
    """Per-core input maps (fallback / external-harness path)."""
    g = _prep_global(query, key, value, Wq, bq, Wk, bk, Wv, bv)
    in_maps = []
    for c in range(NCORES):
        sl = slice(c * HPC, (c + 1) * HPC)
        in_maps.append(dict(
            xq=np.ascontiguousarray(g["xq"][sl]),
            xk=np.ascontiguousarray(g["xk"][sl]),
            vp=np.ascontiguousarray(g["vp"][sl]),
            m65=np.ascontiguousarray(g["m65"][0:128])))
    return in_maps


def _make_runner(nc):
    """Cached sharded-jit runner; donated output buffers created on device."""
    import jax
    import jax.numpy as jnp
    from jax.sharding import Mesh, PartitionSpec, NamedSharding
    from jax.experimental.shard_map import shard_map
    from concourse import mybir
    from concourse.bass2jax import (_bass_exec_p, partition_id_tensor,
                                    install_neuronx_cc_hook)

    install_neuronx_cc_hook()
    partition_name = (nc.partition_id_tensor.name
                      if nc.partition_id_tensor else None)
    in_names, out_names, out_shapes, out_dtypes = [], [], [], []
    for alloc in nc.m.functions[0].allocations:
        if not isinstance(alloc, mybir.MemoryLocationSet):
            continue
        name = alloc.memorylocations[0].name
        if alloc.kind == "ExternalInput":
            if name != partition_name:
                in_names.append(name)
        elif alloc.kind == "ExternalOutput":
            out_names.append(name)
            out_shapes.append(tuple(alloc.tensor_shape))
            out_dtypes.append(mybir.dt.np(alloc.dtype))

    n_params = len(in_names)
    n_outs = len(out_names)
    out_avals = [jax.core.ShapedArray(s, d)
                 for s, d in zip(out_shapes, out_dtypes)]
    all_in_names = tuple(in_names + out_names +
                         ([partition_name] if partition_name else []))
    donate = tuple(range(n_params, n_params + n_outs))

    def _body(*args):
        operands = list(args)
        if partition_name is not None:
            operands.append(partition_id_tensor())
        outs = _bass_exec_p.bind(
            *operands, out_avals=tuple(out_avals), in_names=all_in_names,
            out_names=tuple(out_names), lowering_input_output_aliases=(),
            sim_require_finite=True, sim_require_nnan=True, nc=nc)
        return tuple(outs)

    devices = jax.devices()[:NCORES]
    mesh = Mesh(np.asarray(devices), ("core",))
    in_specs = (PartitionSpec("core"),) * (n_params + n_outs)
    out_specs = (PartitionSpec("core"),) * n_outs
    sharded = jax.jit(
        shard_map(_body, mesh=mesh, in_specs=in_specs,
                  out_specs=out_specs, check_rep=False),
        donate_argnums=donate, keep_unused=True)

    shard = NamedSharding(mesh, PartitionSpec("core"))
    zeros_fn = jax.jit(
        lambda: tuple(jnp.zeros((NCORES * s[0], *s[1:]), d)
                      for s, d in zip(out_shapes, out_dtypes)),
        out_shardings=(shard,) * n_outs)

    def run(in_maps):
        if isinstance(in_maps, dict):  # fast path: pre-concatenated globals
            concat_in = [in_maps[name] for name in in_names]
        else:
            concat_in = [
                np.concatenate([np.asarray(in_maps[c][name])
                                for c in range(NCORES)], axis=0)
                for name in in_names]
        outs = sharded(*concat_in, *zeros_fn())
        np_outs = [np.asarray(o) for o in outs]
        return [
            {name: np_outs[i].reshape(NCORES, *out_shapes[i])[c]
             for i, name in enumerate(out_names)}
            for c in range(NCORES)]

    return run


def _postprocess(res, Wv, bv):
    """res: per-core result dicts -> full [B, S, H, D] f32 output."""
    f32 = np.float32
    Wv, bv = np.asarray(Wv, f32), np.asarray(bv, f32)
    bits = np.stack([np.asarray(res[c]["out"]).view(np.uint16)
                     for c in range(NCORES)])          # [8, 4, 65, S] u16
    outs = (bits.astype(np.uint32) << 16).view(f32)    # bf16 -> f32, exact
    outs = outs.reshape(B * H, 65, S)
    U = outs[:, 0:64, :]                               # [bh, d, s]
    den = outs[:, 64:65, :]                            # [bh, 1, s]
    oT = np.matmul(Wv, U)                              # [bh, e, s] (BLAS)
    oT /= den
    oT += bv[:, None]
    o = oT.reshape(B, H, D, S).transpose(0, 3, 1, 2)   # [B, S, H, D]
    return np.ascontiguousarray(o)


def kernel(query, key, value, Wq, bq, Wk, bk, Wv, bv):
    if "nc" not in _cache:
        _cache["nc"] = _build()
    nc = _cache["nc"]

    g = _prep_global(query, key, value, Wq, bq, Wk, bk, Wv, bv)
    try:
        if "run" not in _cache:
            _cache["run"] = _make_runner(nc)
        res = _cache["run"](g)
    except Exception:
        from concourse.bass_utils import run_bass_kernel_spmd
        in_maps = [
            {k: np.ascontiguousarray(v[c * HPC:(c + 1) * HPC])
             if k != "m65" else np.ascontiguousarray(v[0:128])
             for k, v in g.items()}
            for c in range(NCORES)]
        res = run_bass_kernel_spmd(nc, in_maps, list(range(NCORES))).results
    return _postprocess(res, Wv, bv)


# revision 9
# speedup vs baseline: 35.1541x; 3.8420x over previous
"""Trainium2 Bass kernel for nn_AttentionModel (B=4, S=2048, H=8, D=64).

Sharding: 32 (batch, head) pairs split 4-per-core across 8 NeuronCores
(data + head parallel); the tiny 64x64 projections are folded into
host-side constants, so each core runs pure per-head attention.

Device pipeline (all matmuls bf16, K=128 zero-padded, N=512):
  scores = x̃q^T M̃ x̃k  with  M̃ = [[Wq^T Wk, Wq^T bk], [bq^T Wk, bq.bk]]
  (65x65, host-computed, shared across heads; x̃ = [x; 1]) — this folds
  BOTH q/k projections + biases into one stationary operand.
  ỹ^T[h] = M̃^T x̃q^T[h]  (PE) -> SBUF bf16 [65, S]-in-[128, S] (padded).
  scores^T[j, i] = x̃k^T_jtile^T @ ỹ^T   (PE, zero-padded K).
  exp(s/8) fused with PSUM evacuation: one wide ACT Exp per PSUM strip.
  PV: stationary [x_v | 1] j-tiles -> acc[65, S] in PSUM accumulates
  U^T with the softmax denominator in row 64; one wide ACT copy/head.
  Output ships unnormalized U^T|den [h, 65, S] bf16; the host applies
  the 64x64 V projection + bias, divides by den, and transposes.
"""
import numpy as np

B, S, H, D = 4, 2048, 8, 64
NCORES = 8
HPC = 4            # heads per core
NJ = 16            # key tiles of 128
IC = 512           # query-chunk width
NCH = S // IC      # 4 chunks
EXPW = 1024        # exp strip width (PSUM banks per strip = EXPW/512)
NBUF = 2           # strip double-buffering

_cache = {}


def _build(repeat=1):
    import concourse.bacc as bacc
    import concourse.mybir as mybir
    from concourse.tile import TileContext
    from concourse.bass import ts

    F32 = mybir.dt.float32
    BF16 = mybir.dt.bfloat16
    AF = mybir.ActivationFunctionType

    nc = bacc.Bacc("TRN2", target_bir_lowering=False, debug=False,
                   num_devices=NCORES)

    xq = nc.declare_dram_parameter("xq", [HPC, 65, S], BF16, isOutput=False)
    xk = nc.declare_dram_parameter("xk", [HPC, 65, S], BF16, isOutput=False)
    vp = nc.declare_dram_parameter("vp", [HPC, 128, NJ * 65], BF16,
                                   isOutput=False)
    m65 = nc.declare_dram_parameter("m65", [128, 65], BF16, isOutput=False)
    out_dr = nc.declare_dram_parameter("out", [HPC, 65, S], BF16,
                                       isOutput=True)

    NSTRIP = EXPW // IC            # j-tiles (matmuls) per exp strip
    NROUND = NJ // NSTRIP          # exp strips per (head, chunk)

    with TileContext(nc) as tc:
        with (
            tc.tile_pool(name="constp", bufs=1) as constp,
            tc.tile_pool(name="esbp", bufs=2) as esbp,
            tc.tile_pool(name="psbig", bufs=NBUF, space="PSUM") as psbig,
            tc.tile_pool(name="psacc", bufs=1, space="PSUM") as psacc,
        ):
            xq_sb, xk_sb, vp_sb, y_sb, u_sb = [], [], [], [], []
            for h in range(HPC):
                t = constp.tile([128, S], BF16, name=f"xq{h}")
                nc.gpsimd.memset(t[64:128, :], 0.0)
                xq_sb.append(t)
                t = constp.tile([128, S], BF16, name=f"xk{h}")
                nc.gpsimd.memset(t[64:128, :], 0.0)
                xk_sb.append(t)
                t = constp.tile([128, NJ * 65], BF16, name=f"vp{h}")
                vp_sb.append(t)
                t = constp.tile([128, S], BF16, name=f"y{h}")
                nc.gpsimd.memset(t[64:128, :], 0.0)
                y_sb.append(t)
                t = constp.tile([65, S], BF16, name=f"u{h}")
                u_sb.append(t)
            m65_sb = constp.tile([128, 65], BF16, name="m65")

            for rep in range(repeat):
                # per-call input loads: every tensor split into column
                # halves issued on both HWDGE rings in parallel
                for h in range(HPC):
                    for g, eng in enumerate((nc.sync, nc.scalar)):
                        eng.dma_start(xq_sb[h][0:65, g * 1024:(g + 1) * 1024],
                                      xq[h, :, g * 1024:(g + 1) * 1024])
                        eng.dma_start(xk_sb[h][0:65, g * 1024:(g + 1) * 1024],
                                      xk[h, :, g * 1024:(g + 1) * 1024])
                        eng.dma_start(vp_sb[h][:, g * 520:(g + 1) * 520],
                                      vp[h, :, g * 520:(g + 1) * 520])
                nc.scalar.dma_start(m65_sb[:], m65[:, :])

                # ---- projections: ỹ^T[h] = m65.T @ x̃q^T[h] ----
                for h in range(HPC):
                    pp = psacc.tile([65, S], F32,
                                    name=f"pp{h}_{rep}", tag="acc")
                    for c in range(NCH):
                        nc.tensor.matmul(pp[:, ts(c, IC)], m65_sb[:],
                                         xq_sb[h][:, ts(c, IC)],
                                         start=True, stop=True)
                    nc.vector.tensor_copy(y_sb[h][0:65, :], pp[:])

                # ---- attention ----
                for h in range(HPC):
                    acc4 = psacc.tile([65, S], F32,
                                      name=f"acc{h}_{rep}", tag="acc")
                    for c in range(NCH):
                        esb = esbp.tile([128, NJ * IC], BF16,
                                        name=f"esb{h}_{c}_{rep}", tag="esb")
                        for r in range(NROUND):
                            bigp = psbig.tile([128, EXPW], F32,
                                              name=f"bp{h}_{c}_{r}_{rep}",
                                              tag="bigp")
                            for u in range(NSTRIP):
                                jt = r * NSTRIP + u
                                nc.tensor.matmul(
                                    bigp[:, ts(u, IC)],
                                    xk_sb[h][:, ts(jt, 128)],
                                    y_sb[h][:, ts(c, IC)],
                                    start=True, stop=True)
                            nc.scalar.activation(esb[:, ts(r, EXPW)],
                                                 bigp[:], AF.Exp, scale=0.125)
                        for jt in range(NJ):
                            nc.tensor.matmul(acc4[:, ts(c, IC)],
                                             vp_sb[h][:, jt * 65:jt * 65 + 65],
                                             esb[:, ts(jt, IC)],
                                             start=(jt == 0),
                                             stop=(jt == NJ - 1))
                    nc.vector.tensor_copy(u_sb[h][:, :], acc4[:])
                    eng = nc.sync if h % 2 == 0 else nc.scalar
                    eng.dma_start(out_dr[h, :, :], u_sb[h][:])

    nc.compile()
    return nc


def _bf16_bits(x):
    """f32 ndarray -> uint16 bf16 bits, round-to-nearest-even (vectorized)."""
    u = np.ascontiguousarray(x, np.float32).view(np.uint32)
    return ((u + 0x7FFF + ((u >> 16) & 1)) >> 16).astype(np.uint16)


_ONE_BF16 = np.uint16(0x3F80)


def _prep_global(query, key, value, Wq, bq, Wk, bk, Wv, bv):
    """Host-side layout prep -> global (cores-concatenated) input arrays.

    All bf16 tensors are built as uint16 bit patterns (fast vectorized
    cast) and viewed as ml_dtypes.bfloat16 at the end.
    """
    import ml_dtypes
    BF = ml_dtypes.bfloat16
    f32 = np.float32

    qu = _bf16_bits(query)                 # [B, S, H, D] u16
    ku = _bf16_bits(key)
    vu = _bf16_bits(value)
    Wq, bq = np.asarray(Wq, f32), np.asarray(bq, f32)
    Wk, bk = np.asarray(Wk, f32), np.asarray(bk, f32)

    def padT(xu):  # [B,S,H,D] u16 -> x̃^T [bh, 65, S] u16
        o = np.empty((B * H, 65, S), np.uint16)
        o[:, 0:64, :] = xu.transpose(0, 2, 3, 1).reshape(B * H, D, S)
        o[:, 64, :] = _ONE_BF16
        return o

    xqt, xkt = padT(qu), padT(ku)

    # vp [bh, 128, NJ*65]: [x_v | 1] per j-tile, partition-major
    vpk = np.empty((B * H, 128, NJ, 65), np.uint16)
    vpk[:, :, :, 0:64] = vu.reshape(B, NJ, 128, H, D) \
        .transpose(0, 3, 2, 1, 4).reshape(B * H, 128, NJ, D)
    vpk[:, :, :, 64] = _ONE_BF16
    vpk = vpk.reshape(B * H, 128, NJ * 65)

    m = np.zeros((128, 65), f32)
    m[0:64, 0:64] = Wq.T @ Wk
    m[0:64, 64] = Wq.T @ bk
    m[64, 0:64] = bq @ Wk
    m[64, 64] = bq @ bk
    m65 = _bf16_bits(m).view(BF)

    # global (concatenated-over-cores) arrays: per-core shard c is rows
    # [c*HPC:(c+1)*HPC] — already contiguous, no per-core copies needed
    return dict(xq=xqt.view(BF), xk=xkt.view(BF), vp=vpk.view(BF),
                m65=np.ascontiguousarray(np.tile(m65, (NCORES, 1))))


def _prep_inputs(query, key, value, Wq, bq, Wk, bk, Wv, bv):
    """Per-core input maps (fallback / external-harness path)."""
    g = _prep_global(query, key, value, Wq, bq, Wk, bk, Wv, bv)
    in_maps = []
    for c in range(NCORES):
        sl = slice(c * HPC, (c + 1) * HPC)
        in_maps.append(dict(
            xq=np.ascontiguousarray(g["xq"][sl]),
            xk=np.ascontiguousarray(g["xk"][sl]),
            vp=np.ascontiguousarray(g["vp"][sl]),
            m65=np.ascontiguousarray(g["m65"][0:128])))
    return in_maps


def _make_runner(nc):
    """Cached sharded-jit runner; donated output buffers created on device."""
    import jax
    import jax.numpy as jnp
    from jax.sharding import Mesh, PartitionSpec, NamedSharding
    from jax.experimental.shard_map import shard_map
    from concourse import mybir
    from concourse.bass2jax import (_bass_exec_p, partition_id_tensor,
                                    install_neuronx_cc_hook)

    install_neuronx_cc_hook()
    partition_name = (nc.partition_id_tensor.name
                      if nc.partition_id_tensor else None)
    in_names, out_names, out_shapes, out_dtypes = [], [], [], []
    for alloc in nc.m.functions[0].allocations:
        if not isinstance(alloc, mybir.MemoryLocationSet):
            continue
        name = alloc.memorylocations[0].name
        if alloc.kind == "ExternalInput":
            if name != partition_name:
                in_names.append(name)
        elif alloc.kind == "ExternalOutput":
            out_names.append(name)
            out_shapes.append(tuple(alloc.tensor_shape))
            out_dtypes.append(mybir.dt.np(alloc.dtype))

    n_params = len(in_names)
    n_outs = len(out_names)
    out_avals = [jax.core.ShapedArray(s, d)
                 for s, d in zip(out_shapes, out_dtypes)]
    all_in_names = tuple(in_names + out_names +
                         ([partition_name] if partition_name else []))
    donate = tuple(range(n_params, n_params + n_outs))

    def _body(*args):
        operands = list(args)
        if partition_name is not None:
            operands.append(partition_id_tensor())
        outs = _bass_exec_p.bind(
            *operands, out_avals=tuple(out_avals), in_names=all_in_names,
            out_names=tuple(out_names), lowering_input_output_aliases=(),
            sim_require_finite=True, sim_require_nnan=True, nc=nc)
        return tuple(outs)

    devices = jax.devices()[:NCORES]
    mesh = Mesh(np.asarray(devices), ("core",))
    in_specs = (PartitionSpec("core"),) * (n_params + n_outs)
    out_specs = (PartitionSpec("core"),) * n_outs
    sharded = jax.jit(
        shard_map(_body, mesh=mesh, in_specs=in_specs,
                  out_specs=out_specs, check_rep=False),
        donate_argnums=donate, keep_unused=True)

    shard = NamedSharding(mesh, PartitionSpec("core"))
    zeros_fn = jax.jit(
        lambda: tuple(jnp.zeros((NCORES * s[0], *s[1:]), d)
                      for s, d in zip(out_shapes, out_dtypes)),
        out_shardings=(shard,) * n_outs)

    def run(in_maps):
        if isinstance(in_maps, dict):  # fast path: pre-concatenated globals
            concat_in = [in_maps[name] for name in in_names]
        else:
            concat_in = [
                np.concatenate([np.asarray(in_maps[c][name])
                                for c in range(NCORES)], axis=0)
                for name in in_names]
        outs = sharded(*concat_in, *zeros_fn())
        np_outs = [np.asarray(o) for o in outs]
        return [
            {name: np_outs[i].reshape(NCORES, *out_shapes[i])[c]
             for i, name in enumerate(out_names)}
            for c in range(NCORES)]

    return run


def _postprocess(res, Wv, bv):
    """res: per-core result dicts -> full [B, S, H, D] f32 output."""
    f32 = np.float32
    Wv, bv = np.asarray(Wv, f32), np.asarray(bv, f32)
    bits = np.stack([np.asarray(res[c]["out"]).view(np.uint16)
                     for c in range(NCORES)])          # [8, 4, 65, S] u16
    outs = (bits.astype(np.uint32) << 16).view(f32)    # bf16 -> f32, exact
    outs = outs.reshape(B * H, 65, S)
    U = outs[:, 0:64, :]                               # [bh, d, s]
    den = outs[:, 64:65, :]                            # [bh, 1, s]
    oT = np.matmul(Wv, U)                              # [bh, e, s] (BLAS)
    oT /= den
    oT += bv[:, None]
    o = oT.reshape(B, H, D, S).transpose(0, 3, 1, 2)   # [B, S, H, D]
    return np.ascontiguousarray(o)


def kernel(query, key, value, Wq, bq, Wk, bk, Wv, bv):
    if "nc" not in _cache:
        _cache["nc"] = _build()
    nc = _cache["nc"]

    g = _prep_global(query, key, value, Wq, bq, Wk, bk, Wv, bv)
    try:
        if "run" not in _cache:
            _cache["run"] = _make_runner(nc)
        res = _cache["run"](g)
    except Exception:
        from concourse.bass_utils import run_bass_kernel_spmd
        in_maps = [
            {k: np.ascontiguousarray(v[c * HPC:(c + 1) * HPC])
             if k != "m65" else np.ascontiguousarray(v[0:128])
             for k, v in g.items()}
            for c in range(NCORES)]
        res = run_bass_kernel_spmd(nc, in_maps, list(range(NCORES))).results
    return _postprocess(res, Wv, bv)
